# revision 32
# baseline (speedup 1.0000x reference)
"""MoE grouped-linear (ragged matmul + bias) on 8 TRN2 NeuronCores.

Expert-parallel sharding: core e computes tokens of expert e:
    out_e = X_e[cap, 2048] @ W_e[2048, 8192] + bias
Tokens are pre-sorted by expert (contiguous groups), so the "all-to-all"
is a free host-side slice/concat. No on-device collectives.

Production path: _build4 (A/B/C segments), bf16 matmuls, bf16 bias and
bf16 outputs (rel err 3.9e-3 vs the 2e-2 gate). Structure, from the
TimelineSim cost-model (tsim_tool.py) + HW reps-slope measurements:

- Per-core tensor floor = 1,048,576 PE cycles = 436.9us at 2.4GHz; the
  graded baseline (452.2us) was floor + cold-start DMA stalls + drain.
- Segment A (cols 0..1023): k-outer over mi-groups of 4, nblk=2 -> the
  first sweep consumes W at ~150GB/s (vs mi-outer's 600 demand against
  ~352GB/s supply), killing the block-0 stall. HW G-sweep microbench
  (_build_ldw) showed bf16 LDWEIGHTS is fully hidden at G=1/2/4 (the
  documented 107ns exposure was fp32r-specific), so nblk=2 is free.
- Segments B1..B3 (cols 1024..7167, bw=2048, nblk=4, mi-outer): W for
  block b+1 prefetches into the second w_pool buffer during block b.
- Segment C (cols 7168..8191, nblk=2): last mi runs chains of width
  512/256/256 sequentially so the drain is one [P,256] evict + 64KB DMA
  (~3.6us vs 5.6 for 4 simultaneous chains).
- Cold window: all A-era loads ride the SP queue in one deadline-ordered
  FIFO (W k-slices just-in-time at 1.71us cadence, xt tiers and biasA
  interleaved in the slack); only xt0a loads in parallel on gpsimd. 5
  warmup matmuls on a DVE-zeroed scratch tile ramp the PE clock during
  the cold open.
- DMA scheduling: the simulator's DMA device round-robins *ready*
  transfers from all queues and HWDGE does NOT hold a queue on a
  descriptor's semaphore waits, so queue back-pressure cannot throttle
  loads. Ordering is done by FIFO position instead: early-critical
  loads (xt0..3 in 3 k-tiers, then xt4..7 + biasA, all ordered by first
  need) on gpsimd; per-block bias chunks ride the SP queue AFTER that
  block's W slices. Out-DMAs ride the Activation queue; o_pool bufs=8
  so evictions never wait on out-DMA completion.
- W k0 is chunked 4x512 cols and xt0..3 split at k=2/8 so the first
  matmul gates on ~190KB (~3.4us cold open, latency-dominated).
- PE warmup matmuls during the cold open are NOT useful: the cold
  window is DMA-bound, so slow-clock (p-state ramp) matmuls there are
  free anyway (verified in-model: warmup was net-neutral).

Dead ends, measured on HW: fp8 e4m3 DoubleRow — _build_dr microbench
(2048 DoubleRow insts, 2-plane lhsT/rhs) times like bf16 (~1.0 cycles
per output row, i.e. 2x MACs via the 2-plane contraction), NOT the
0.5 cyc/row the CoreSim cost model charges (which would be 4x). So a
plane-packed 3-term hi/lo scheme (x_hi@w_hi + x_lo@w_hi + x_hi@w_lo,
~1% rel err) costs 1.5x bf16 — slower, confirming the prior session:
single-pass fp8 4.0e-2 / 2-pass 2.7e-2 both fail the 2e-2 gate and
1.5-pass-equivalent packing does not exist at 2x. fp32->bf16-out adds
~1.5e-3 rel err (gate 2e-2) and halves out-DMA. Under tenant/thermal
contention all variants converge to ~540-585us (power-bound regime);
slope measurements need min-of-many-rounds filtering (see test.py).

Second iteration round (same session): the cold-window DMA races were
eliminated by putting the ENTIRE A-window load sequence on the SP
queue in deadline order (single-queue FIFO is deterministic; the
shared DMA device round-robins across queues, so multi-queue splits
of urgent loads always raced). With the PE then the binding resource
from t~3.5us, a p-state warmup became profitable: DVE memsets a
scratch tile at t~0 and 5 dummy matmuls fill the cold open, so the
3us clock ramp completes before real matmuls (real-MM busy hits the
436.7us floor). Tried and rejected: A at 512 cols nblk=1 all-mi
k-outer (concentrates all 8 xt tiles into the first window - worse);
alternating the cold schedule across SP+Act for 2x dispatch rate
(reintroduces device races - worse).

TimelineSim totals (single shot): _build2 474.9us -> _build4 445.7us
= 0.1 + 1.5 (memset wait) + 438.8 busy (incl 2.1us warmup; real MMs
at the 436.7 floor) + ~1.7 residual jitter + 3.6 drain. HW
quiet-window slope ~430-440us for all variants (noise swamps deltas);
the graded single-shot metric should capture the ~25us modeled
improvement's real-HW share (~12-15us).
"""

import numpy as np

E, IN, OUT = 8, 2048, 8192
P = 128
NTILE = 512

_cache = {}


def _build(cap, dtype_name="float32r", reps=1, mode="full", ntile=None):
    import contextlib

    import concourse.mybir as mybir
    import concourse.tile as tile
    from concourse import bacc

    mm_dt = getattr(mybir.dt, dtype_name)
    nt = ntile or NTILE
    KT = IN // P            # 16 k-tiles
    MT = cap // P           # m-tiles per core
    NT = OUT // nt          # n-tiles

    nc = bacc.Bacc(None, target_bir_lowering=False, debug=False)
    with tile.TileContext(nc) as tc:
        with tc.tile_pool(name="dram", bufs=1, space="DRAM") as dram:
            # xt_d[mi, p, k, j] = X[mi*P + j, k*P + p] — per-mi contiguous
            # 1MB slices so the first matmul group can start after ~1MB of DMA
            xt_d = dram.tile((MT, P, KT, P), mm_dt, kind="ExternalInput")
            w_d = dram.tile((P, KT, OUT), mm_dt, kind="ExternalInput")
            bias_d = dram.tile((P, OUT), mybir.dt.float32, kind="ExternalInput")
            out_d = dram.tile((P, MT, OUT), mybir.dt.float32, kind="ExternalOutput")

            with tc.tile_pool(name="resident", bufs=1) as res_pool, \
                 tc.tile_pool(name="wchunk", bufs=2) as w_pool, \
                 tc.tile_pool(name="evict", bufs=6) as o_pool, \
                 tc.tile_pool(name="acc", bufs=(3 if nt > 512 else 6), space="PSUM") as ps_pool:
                loop = tc.For_i(0, reps, 1) if reps > 1 else contextlib.nullcontext()
                with loop:
                    # W stream owns the sync queue; X^T + bias load in
                    # parallel on the gpsimd queue, first-needed first.
                    w_sbs = [None] * NT
                    w_sbs[0] = w_pool.tile([P, KT, nt], mm_dt, tag="w",
                                           name="w_sb0")
                    nc.sync.dma_start(w_sbs[0][:], w_d[:, :, 0:nt])

                    xt_sb = [res_pool.tile([P, KT, P], mm_dt, tag=f"xt{mi}",
                                           name=f"xt_sb{mi}")
                             for mi in range(MT)]
                    bias_sb = res_pool.tile([P, OUT], mybir.dt.float32)
                    nc.gpsimd.dma_start(xt_sb[0][:], xt_d[0])
                    nc.gpsimd.dma_start(bias_sb[:], bias_d[:])
                    for mi in range(1, MT):
                        nc.gpsimd.dma_start(xt_sb[mi][:], xt_d[mi])

                    for ni in range(NT):
                        w_sb = w_sbs[ni]
                        if w_sb is None and mode in ("mm_only", "same_w"):
                            w_sb = w_sbs[0]
                        elif w_sb is None:
                            w_sb = w_pool.tile([P, KT, nt], mm_dt, tag="w",
                                               name=f"w_sb{ni}")
                            nc.sync.dma_start(
                                w_sb[:], w_d[:, :, ni * nt:(ni + 1) * nt])
                        for mi in range(MT):
                            ps = ps_pool.tile([P, nt], mybir.dt.float32)
                            for k in range(KT):
                                nc.tensor.matmul(
                                    ps[:],
                                    lhsT=xt_sb[0][:, 0, :] if mode == "same_w"
                                    else xt_sb[mi][:, k, :],
                                    rhs=w_sb[:, k, :],
                                    start=(k == 0),
                                    stop=(k == KT - 1),
                                )
                            if mode in ("mm_only", "same_w") and not (ni == NT - 1 and mi == MT - 1):
                                continue
                            o_sb = o_pool.tile([P, nt], mybir.dt.float32)
                            nc.vector.tensor_add(
                                out=o_sb[:], in0=ps[:],
                                in1=bias_sb[:, ni * nt:(ni + 1) * nt])
                            nc.sync.dma_start(
                                out_d[:, mi, ni * nt:(ni + 1) * nt], o_sb[:])
    nc.compile()
    names = dict(xt=xt_d.name, w=w_d.name, bias=bias_d.name, out=out_d.name)
    return nc, names


def _get(cap, dtype_name="float32r", reps=1, mode="full", ntile=None):
    key = (cap, dtype_name, reps, mode, ntile)
    if key not in _cache:
        _cache[key] = _build(cap, dtype_name, reps, mode, ntile)
    return _cache[key]


def _build2(cap, dtype_name="bfloat16", reps=1, nblk=4, psum_bufs=8,
            mode="full"):
    """LDW-amortized variant: loop (ni_blk, mi, k, ni-in-blk) so each
    stationary x^T[mi,k] serves `nblk` consecutive 512-col matmuls.
    W is streamed once, in [128, KT, nblk*512] blocks, per-k-slice DMAs.
    mode: full | no_evict (only last gen evicts) | same_w (fixed stationary)
    """
    import contextlib

    import concourse.mybir as mybir
    import concourse.tile as tile
    from concourse import bacc

    mm_dt = getattr(mybir.dt, dtype_name)
    nt = NTILE                      # 512
    KT = IN // P                    # 16
    MT = cap // P                   # m-tiles
    NBLK = OUT // (nblk * nt)       # blocks of nblk n-tiles
    bw = nblk * nt                  # block width in cols

    nc = bacc.Bacc(None, target_bir_lowering=False, debug=False)
    with tile.TileContext(nc) as tc:
        with tc.tile_pool(name="dram", bufs=1, space="DRAM") as dram:
            xt_d = dram.tile((MT, P, KT, P), mm_dt, kind="ExternalInput")
            w_d = dram.tile((P, KT, OUT), mm_dt, kind="ExternalInput")
            bias_d = dram.tile((P, OUT), mybir.dt.float32, kind="ExternalInput")
            out_d = dram.tile((P, MT, OUT), mybir.dt.float32, kind="ExternalOutput")

            with tc.tile_pool(name="resident", bufs=1) as res_pool, \
                 tc.tile_pool(name="wblk", bufs=2) as w_pool, \
                 tc.tile_pool(name="evict", bufs=4) as o_pool, \
                 tc.tile_pool(name="acc", bufs=psum_bufs, space="PSUM") as ps_pool:
                loop = tc.For_i(0, reps, 1) if reps > 1 else contextlib.nullcontext()
                with loop:
                    w_sbs = [None] * NBLK
                    w_sbs[0] = w_pool.tile([P, KT, bw], mm_dt, tag="w",
                                           name="w_sb0")
                    # per-k-slice DMAs so the first matmul is gated on
                    # one k-slice, not the whole 8MB block
                    for k in range(KT):
                        nc.sync.dma_start(w_sbs[0][:, k], w_d[:, k, 0:bw])

                    xt_sb = [res_pool.tile([P, KT, P], mm_dt, tag=f"xt{mi}",
                                           name=f"xt_sb{mi}")
                             for mi in range(MT)]
                    bias_sb = res_pool.tile([P, OUT], mybir.dt.float32)
                    nc.gpsimd.dma_start(xt_sb[0][:], xt_d[0])
                    nc.gpsimd.dma_start(bias_sb[:], bias_d[:])
                    for mi in range(1, MT):
                        nc.gpsimd.dma_start(xt_sb[mi][:], xt_d[mi])

                    for blk in range(NBLK):
                        w_sb = w_sbs[blk]
                        if w_sb is None:
                            w_sb = w_pool.tile([P, KT, bw], mm_dt, tag="w",
                                               name=f"w_sb{blk}")
                            for k in range(KT):
                                nc.sync.dma_start(
                                    w_sb[:, k],
                                    w_d[:, k, blk * bw:(blk + 1) * bw])
                        for mi in range(MT):
                            pss = [ps_pool.tile([P, nt], mybir.dt.float32,
                                                tag="ps",
                                                name=f"ps{blk}_{mi}_{j}")
                                   for j in range(nblk)]
                            for k in range(KT):
                                for ni in range(nblk):
                                    nc.tensor.matmul(
                                        pss[ni][:],
                                        lhsT=xt_sb[0][:, 0, :] if mode == "same_w"
                                        else xt_sb[mi][:, k, :],
                                        rhs=w_sb[:, k, ni * nt:(ni + 1) * nt],
                                        start=(k == 0),
                                        stop=(k == KT - 1),
                                    )
                            if mode in ("no_evict", "same_w") and not (
                                    blk == NBLK - 1 and mi == MT - 1):
                                continue
                            for ni in range(nblk):
                                o_sb = o_pool.tile([P, nt], mybir.dt.float32)
                                col0 = blk * bw + ni * nt
                                nc.vector.tensor_add(
                                    out=o_sb[:], in0=pss[ni][:],
                                    in1=bias_sb[:, col0:col0 + nt])
                                nc.sync.dma_start(
                                    out_d[:, mi, col0:col0 + nt], o_sb[:])
    nc.compile()
    names = dict(xt=xt_d.name, w=w_d.name, bias=bias_d.name, out=out_d.name)
    return nc, names


def _get2(cap, dtype_name="bfloat16", reps=1, nblk=4, psum_bufs=8,
          mode="full"):
    key = ("v2", cap, dtype_name, reps, nblk, psum_bufs, mode)
    if key not in _cache:
        _cache[key] = _build2(cap, dtype_name, reps, nblk, psum_bufs, mode)
    return _cache[key]


def _build3(cap, reps=1, out_dt_name="bfloat16"):
    """Cold-start-optimized variant.

    Block A (first 2048 cols): k-outer over mi-pairs so the first sweep
    consumes W k-slices at ~300GB/s (supply ~350) instead of mi-outer's
    600GB/s — kills the block-0 DMA stall. G=4 stationary reuse kept
    (each xt[mi,k] serves ni0..3); 2mi x 4ni = 8 live PSUM banks.
    Blocks B1..B3 (cols 2048..8191): mi-outer as _build2 (prefetched).
    Out DMAs ride the Activation queue so they never block the SP
    queue's W prefetch; non-critical loads (xt2b/3b, xt4..7, biasB) are
    emitted on the Act queue BETWEEN eviction DMAs, so the out-DMAs'
    semaphore waits throttle them until the cold-start window is over.
    bias is bf16 [P, OUT]; outputs are stored bf16 (abs err +<=0.011 vs
    gate 0.114). First matmul gated on ~190KB: W k0 in 4 chunks, xt0/1
    split at k0..1. Tail: last mi's 4 out-DMAs split across SP + Act.
    """
    import contextlib

    import concourse.mybir as mybir
    import concourse.tile as tile
    from concourse import bacc

    mm_dt = mybir.dt.bfloat16
    out_dt = getattr(mybir.dt, out_dt_name)
    nt = NTILE                      # 512
    KT = IN // P                    # 16
    MT = cap // P                   # m-tiles
    nblk = 4
    bw = nblk * nt                  # 2048
    NBLK = OUT // bw                # 4 (A + 3 B-blocks)
    GM = min(2, MT)                 # mi-group size in block A
    tuned = MT == 8                 # DMA schedule tuned for cap=1024

    nc = bacc.Bacc(None, target_bir_lowering=False, debug=False)
    with tile.TileContext(nc) as tc:
        with tc.tile_pool(name="dram", bufs=1, space="DRAM") as dram:
            xt_d = dram.tile((MT, P, KT, P), mm_dt, kind="ExternalInput")
            w_d = dram.tile((P, KT, OUT), mm_dt, kind="ExternalInput")
            bias_d = dram.tile((P, OUT), mm_dt, kind="ExternalInput")
            out_d = dram.tile((P, MT, OUT), out_dt, kind="ExternalOutput")

            with tc.tile_pool(name="resident", bufs=1) as res_pool, \
                 tc.tile_pool(name="wblk", bufs=2) as w_pool, \
                 tc.tile_pool(name="evict", bufs=8) as o_pool, \
                 tc.tile_pool(name="acc", bufs=8, space="PSUM") as ps_pool:
                loop = tc.For_i(0, reps, 1) if reps > 1 else contextlib.nullcontext()
                with loop:
                    # ---- block A weights: per-k DMAs, k0 chunked x4,
                    # k1 chunked x2 so the first matmuls gate on 128KB
                    wA = w_pool.tile([P, KT, bw], mm_dt, tag="w", name="wA")
                    for c in range(4):
                        nc.sync.dma_start(wA[:, 0, c * nt:(c + 1) * nt],
                                          w_d[:, 0, c * nt:(c + 1) * nt])
                    for c in range(2):
                        nc.sync.dma_start(
                            wA[:, 1, c * 2 * nt:(c + 1) * 2 * nt],
                            w_d[:, 1, c * 2 * nt:(c + 1) * 2 * nt])
                    for k in range(2, KT):
                        nc.sync.dma_start(wA[:, k], w_d[:, k, 0:bw])

                    # ---- early-critical loads on the gpsimd queue
                    xt_sb = [res_pool.tile([P, KT, P], mm_dt, tag=f"xt{mi}",
                                           name=f"xt_sb{mi}")
                             for mi in range(MT)]
                    bias_sb = res_pool.tile([P, OUT], mm_dt)
                    if tuned:
                        for mi in (0, 1):
                            nc.gpsimd.dma_start(xt_sb[mi][:, 0:2],
                                                xt_d[mi][:, 0:2])
                        for mi in (0, 1):
                            nc.gpsimd.dma_start(xt_sb[mi][:, 2:KT],
                                                xt_d[mi][:, 2:KT])
                        nc.gpsimd.dma_start(bias_sb[:, 0:bw], bias_d[:, 0:bw])
                        # later-needed xt tiles ride the SP queue AFTER
                        # block A's W slices: FIFO keeps them off the
                        # cold-start window; B1's prefetch has 80us slack
                        for mi in range(2, MT):
                            nc.sync.dma_start(xt_sb[mi][:], xt_d[mi])
                    else:
                        for mi in range(MT):
                            nc.gpsimd.dma_start(xt_sb[mi][:], xt_d[mi])
                        nc.gpsimd.dma_start(bias_sb[:], bias_d[:])

                    def evict(ps, mi, col0, dma_eng=None):
                        o_sb = o_pool.tile([P, nt], out_dt)
                        nc.vector.tensor_add(
                            out=o_sb[:], in0=ps[:],
                            in1=bias_sb[:, col0:col0 + nt])
                        (dma_eng or nc.scalar).dma_start(
                            out_d[:, mi, col0:col0 + nt], o_sb[:])

                    # ---- block A: k-outer over mi-groups
                    for g in range(0, MT, GM):
                        gm = min(GM, MT - g)
                        pss = [ps_pool.tile([P, nt], mybir.dt.float32,
                                            tag="ps", name=f"psA{g}_{j}")
                               for j in range(gm * nblk)]
                        for k in range(KT):
                            for mj in range(gm):
                                for ni in range(nblk):
                                    nc.tensor.matmul(
                                        pss[mj * nblk + ni][:],
                                        lhsT=xt_sb[g + mj][:, k, :],
                                        rhs=wA[:, k, ni * nt:(ni + 1) * nt],
                                        start=(k == 0),
                                        stop=(k == KT - 1),
                                    )
                        for mj in range(gm):
                            for ni in range(nblk):
                                evict(pss[mj * nblk + ni], g + mj, ni * nt)
                        # throttled loads: queued on Act behind this group's
                        # out-DMAs, so they transfer only after the cold
                        # window; each arrives well before it is needed


                    # ---- blocks B1..B3: mi-outer (W prefetched)
                    for blk in range(1, NBLK):
                        w_sb = w_pool.tile([P, KT, bw], mm_dt, tag="w",
                                           name=f"wB{blk}")
                        for k in range(KT):
                            nc.sync.dma_start(
                                w_sb[:, k], w_d[:, k, blk * bw:(blk + 1) * bw])
                        if tuned:
                            nc.sync.dma_start(
                                bias_sb[:, blk * bw:(blk + 1) * bw],
                                bias_d[:, blk * bw:(blk + 1) * bw])
                        for mi in range(MT):
                            pss = [ps_pool.tile([P, nt], mybir.dt.float32,
                                                tag="ps",
                                                name=f"ps{blk}_{mi}_{j}")
                                   for j in range(nblk)]
                            for k in range(KT):
                                for ni in range(nblk):
                                    nc.tensor.matmul(
                                        pss[ni][:],
                                        lhsT=xt_sb[mi][:, k, :],
                                        rhs=w_sb[:, k, ni * nt:(ni + 1) * nt],
                                        start=(k == 0),
                                        stop=(k == KT - 1),
                                    )
                            last = blk == NBLK - 1 and mi == MT - 1
                            for ni in range(nblk):
                                dq = nc.sync if (last and ni < 2) else None
                                evict(pss[ni], mi, blk * bw + ni * nt,
                                      dma_eng=dq)
    nc.compile()
    names = dict(xt=xt_d.name, w=w_d.name, bias=bias_d.name, out=out_d.name)
    return nc, names


def _build4(cap, reps=1, out_dt_name="bfloat16"):
    """A/B/C-segment variant (requires LDW hidden at G=2, measured on HW).

    A: cols 0..1023, k-outer, GM=4 mi-group, nblk=2 (G=2): W demand
    ~150GB/s in the cold window, 8.25MB of early DMA vs ~9.6MB capacity.
    B1..B3: cols 1024..7167, bw=2048 nblk=4 mi-outer (prefetched).
    C: cols 7168..8191, bw=1024 nblk=2 mi-outer: last mi drains only 2
    chains -> short tail; its 2 out-DMAs split across SP/Act queues.
    """
    import contextlib

    import concourse.mybir as mybir
    import concourse.tile as tile
    from concourse import bacc

    mm_dt = mybir.dt.bfloat16
    out_dt = getattr(mybir.dt, out_dt_name)
    nt = NTILE                      # 512
    KT = IN // P                    # 16
    MT = cap // P
    tuned = MT == 8

    nc = bacc.Bacc(None, target_bir_lowering=False, debug=False)
    with tile.TileContext(nc) as tc:
        with tc.tile_pool(name="dram", bufs=1, space="DRAM") as dram:
            xt_d = dram.tile((MT, P, KT, P), mm_dt, kind="ExternalInput")
            w_d = dram.tile((P, KT, OUT), mm_dt, kind="ExternalInput")
            bias_d = dram.tile((P, OUT), mm_dt, kind="ExternalInput")
            out_d = dram.tile((P, MT, OUT), out_dt, kind="ExternalOutput")

            with tc.tile_pool(name="resident", bufs=1) as res_pool, \
                 tc.tile_pool(name="wblk", bufs=2) as w_pool, \
                 tc.tile_pool(name="evict", bufs=8) as o_pool, \
                 tc.tile_pool(name="acc", bufs=8, space="PSUM") as ps_pool:
                loop = tc.For_i(0, reps, 1) if reps > 1 else contextlib.nullcontext()
                with loop:
                    # ---- segment A weights: [P, KT, 1024], per-k DMAs,
                    # k0 in two 512-col chunks
                    awb = 2 * nt    # 1024
                    wA = w_pool.tile([P, KT, awb], mm_dt, tag="w", name="wA")
                    xt_sb = [res_pool.tile([P, KT, P], mm_dt, tag=f"xt{mi}",
                                           name=f"xt_sb{mi}")
                             for mi in range(MT)]
                    bias_sb = res_pool.tile([P, OUT], mm_dt)

                    def wk(k):
                        nc.sync.dma_start(wA[:, k], w_d[:, k, 0:awb])

                    def xts(mi, k0, k1):
                        nc.sync.dma_start(xt_sb[mi][:, k0:k1],
                                          xt_d[mi][:, k0:k1])

                    if tuned:
                        # Deterministic cold-window schedule: everything on
                        # the SP queue in deadline order (single-queue FIFO
                        # -> no cross-queue round-robin races); only xt0a
                        # loads in parallel on gpsimd for the cold open.
                        # Deadlines: wA k_j at ~3.6+1.71j us; xt[mi] tier
                        # (k0..1 / k2..7 / k8..15) at its first matmul;
                        # xt4..7 by group1 (~31us); biasA by first evict.
                        nc.gpsimd.dma_start(xt_sb[0][:, 0:2], xt_d[0][:, 0:2])
                        for c in range(2):
                            nc.sync.dma_start(
                                wA[:, 0, c * nt:(c + 1) * nt],
                                w_d[:, 0, c * nt:(c + 1) * nt])
                        xts(1, 0, 2)
                        wk(1)
                        xts(2, 0, 2)
                        xts(3, 0, 2)
                        xts(0, 2, 8)
                        wk(2)
                        xts(1, 2, 8)
                        xts(2, 2, 8)
                        xts(3, 2, 8)
                        wk(3)
                        wk(4)
                        wk(5)
                        xts(4, 0, KT)
                        wk(6)
                        xts(0, 8, KT)
                        wk(7)
                        xts(1, 8, KT)
                        wk(8)
                        xts(5, 0, KT)
                        xts(2, 8, KT)
                        xts(3, 8, KT)
                        wk(9)
                        wk(10)
                        wk(11)
                        nc.sync.dma_start(bias_sb[:, 0:awb], bias_d[:, 0:awb])
                        wk(12)
                        xts(6, 0, KT)
                        wk(13)
                        wk(14)
                        wk(15)
                        xts(7, 0, KT)
                    else:
                        for c in range(2):
                            nc.sync.dma_start(
                                wA[:, 0, c * nt:(c + 1) * nt],
                                w_d[:, 0, c * nt:(c + 1) * nt])
                        for k in range(1, KT):
                            wk(k)
                        for mi in range(MT):
                            nc.gpsimd.dma_start(xt_sb[mi][:], xt_d[mi])
                        nc.gpsimd.dma_start(bias_sb[:], bias_d[:])

                    def evict(ps, mi, col0, dma_eng=None):
                        o_sb = o_pool.tile([P, nt], out_dt)
                        nc.vector.tensor_add(
                            out=o_sb[:], in0=ps[:],
                            in1=bias_sb[:, col0:col0 + nt])
                        (dma_eng or nc.scalar).dma_start(
                            out_d[:, mi, col0:col0 + nt], o_sb[:])

                    # PE p-state warmup: DVE zeroes a scratch tile at t~0,
                    # then 5 dummy matmuls fill the cold-open DMA wait
                    # (~3.4us) so the clock ramp (3us to full speed)
                    # completes before the first real matmul.
                    wu_sb = res_pool.tile([P, 640], mm_dt, name="wu")
                    nc.vector.memset(wu_sb[:], 0.0)
                    wu_ps = ps_pool.tile([P, nt], mybir.dt.float32,
                                         tag="ps", name="wu_ps")
                    for _ in range(5):
                        nc.tensor.matmul(
                            wu_ps[:], lhsT=wu_sb[:, 0:P],
                            rhs=wu_sb[:, P:P + nt], start=True, stop=True,
                            skip_group_check=True)

                    # ---- segment A: k-outer, groups of GM=4 mi, nblk=2
                    GM = min(4, MT)
                    for g in range(0, MT, GM):
                        gm = min(GM, MT - g)
                        pss = [ps_pool.tile([P, nt], mybir.dt.float32,
                                            tag="ps", name=f"psA{g}_{j}")
                               for j in range(gm * 2)]
                        for k in range(KT):
                            for mj in range(gm):
                                for ni in range(2):
                                    nc.tensor.matmul(
                                        pss[mj * 2 + ni][:],
                                        lhsT=xt_sb[g + mj][:, k, :],
                                        rhs=wA[:, k, ni * nt:(ni + 1) * nt],
                                        start=(k == 0),
                                        stop=(k == KT - 1),
                                    )
                        for mj in range(gm):
                            for ni in range(2):
                                evict(pss[mj * 2 + ni], g + mj, ni * nt)

                    # ---- segments B: bw=2048, nblk=4, mi-outer
                    nblk = 4
                    bw = nblk * nt
                    nB = (OUT - 2 * awb) // bw      # 3
                    for blk in range(nB):
                        col_b = awb + blk * bw
                        w_sb = w_pool.tile([P, KT, bw], mm_dt, tag="w",
                                           name=f"wB{blk}")
                        for k in range(KT):
                            nc.sync.dma_start(
                                w_sb[:, k], w_d[:, k, col_b:col_b + bw])
                        if tuned:
                            nc.sync.dma_start(bias_sb[:, col_b:col_b + bw],
                                              bias_d[:, col_b:col_b + bw])
                        for mi in range(MT):
                            pss = [ps_pool.tile([P, nt], mybir.dt.float32,
                                                tag="ps",
                                                name=f"ps{blk}_{mi}_{j}")
                                   for j in range(nblk)]
                            for k in range(KT):
                                for ni in range(nblk):
                                    nc.tensor.matmul(
                                        pss[ni][:],
                                        lhsT=xt_sb[mi][:, k, :],
                                        rhs=w_sb[:, k, ni * nt:(ni + 1) * nt],
                                        start=(k == 0),
                                        stop=(k == KT - 1),
                                    )
                            for ni in range(nblk):
                                evict(pss[ni], mi, col_b + ni * nt)

                    # ---- segment C: last 1024 cols, nblk=2, mi-outer
                    col_c = OUT - awb
                    wC = w_pool.tile([P, KT, awb], mm_dt, tag="w", name="wC")
                    for k in range(KT):
                        nc.sync.dma_start(wC[:, k], w_d[:, k, col_c:col_c + awb])
                    if tuned:
                        nc.sync.dma_start(bias_sb[:, col_c:col_c + awb],
                                          bias_d[:, col_c:col_c + awb])
                    for mi in range(MT):
                        last = mi == MT - 1
                        if last:
                            # sequential chains of shrinking width (512,
                            # 256, 256): each eviction hides under the next
                            # chain's matmuls; the drain is one [P,256]
                            # evict + 64KB DMA. (G=1 LDW is free for bf16
                            # per HW measurement.)
                            w0 = 0
                            for j, cw in enumerate((nt, nt // 2, nt // 2)):
                                ps = ps_pool.tile([P, cw], mybir.dt.float32,
                                                  tag="ps",
                                                  name=f"psC_{mi}_{j}")
                                for k in range(KT):
                                    nc.tensor.matmul(
                                        ps[:],
                                        lhsT=xt_sb[mi][:, k, :],
                                        rhs=wC[:, k, w0:w0 + cw],
                                        start=(k == 0),
                                        stop=(k == KT - 1),
                                    )
                                o_sb = o_pool.tile([P, cw], out_dt)
                                nc.vector.tensor_add(
                                    out=o_sb[:], in0=ps[:],
                                    in1=bias_sb[:, col_c + w0:col_c + w0 + cw])
                                (nc.sync if j == 0 else nc.scalar).dma_start(
                                    out_d[:, mi, col_c + w0:col_c + w0 + cw],
                                    o_sb[:])
                                w0 += cw
                            continue
                        pss = [ps_pool.tile([P, nt], mybir.dt.float32,
                                            tag="ps", name=f"psC_{mi}_{j}")
                               for j in range(2)]
                        for k in range(KT):
                            for ni in range(2):
                                nc.tensor.matmul(
                                    pss[ni][:],
                                    lhsT=xt_sb[mi][:, k, :],
                                    rhs=wC[:, k, ni * nt:(ni + 1) * nt],
                                    start=(k == 0),
                                    stop=(k == KT - 1),
                                )
                        for ni in range(2):
                            evict(pss[ni], mi, col_c + ni * nt)
    nc.compile()
    names = dict(xt=xt_d.name, w=w_d.name, bias=bias_d.name, out=out_d.name)
    return nc, names


def _get4(cap, reps=1, out_dt_name="bfloat16"):
    key = ("v4", cap, reps, out_dt_name)
    if key not in _cache:
        _cache[key] = _build4(cap, reps, out_dt_name)
    return _cache[key]


def _build5(cap, reps=1, out_dt_name="bfloat16"):
    """A(512, k-outer, nblk=1, all-mi group) variant.

    G=1 LDW is free for bf16 (HW-measured), so segment A can be one
    512-col k-outer sweep over all 8 mi: W demand 75GB/s (21% device
    duty), leaving 79% for the xt tiers -> no early jitter. B1 is
    k-outer (GM=2, nblk=4, ~300GB/s just-in-time) so it needs no
    prefetch during A. B2/B3 mi-outer (prefetched). C = 1536 cols,
    nblk=3; last mi runs sequential chains 512/512/256/256 so the
    drain is one [P,256] evict + 64KB DMA.
    """
    import contextlib

    import concourse.mybir as mybir
    import concourse.tile as tile
    from concourse import bacc

    mm_dt = mybir.dt.bfloat16
    out_dt = getattr(mybir.dt, out_dt_name)
    nt = NTILE                      # 512
    KT = IN // P                    # 16
    MT = cap // P
    tuned = MT == 8

    nc = bacc.Bacc(None, target_bir_lowering=False, debug=False)
    with tile.TileContext(nc) as tc:
        with tc.tile_pool(name="dram", bufs=1, space="DRAM") as dram:
            xt_d = dram.tile((MT, P, KT, P), mm_dt, kind="ExternalInput")
            w_d = dram.tile((P, KT, OUT), mm_dt, kind="ExternalInput")
            bias_d = dram.tile((P, OUT), mm_dt, kind="ExternalInput")
            out_d = dram.tile((P, MT, OUT), out_dt, kind="ExternalOutput")

            with tc.tile_pool(name="resident", bufs=1) as res_pool, \
                 tc.tile_pool(name="wblk", bufs=2) as w_pool, \
                 tc.tile_pool(name="evict", bufs=8) as o_pool, \
                 tc.tile_pool(name="acc", bufs=8, space="PSUM") as ps_pool:
                loop = tc.For_i(0, reps, 1) if reps > 1 else contextlib.nullcontext()
                with loop:
                    # ---- segment A weights: [P, KT, 512], k0 in 2 chunks
                    wA = w_pool.tile([P, KT, nt], mm_dt, tag="w", name="wA")
                    for c in range(2):
                        nc.sync.dma_start(wA[:, 0, c * 256:(c + 1) * 256],
                                          w_d[:, 0, c * 256:(c + 1) * 256])
                    for k in range(1, KT):
                        nc.sync.dma_start(wA[:, k], w_d[:, k, 0:nt])

                    xt_sb = [res_pool.tile([P, KT, P], mm_dt, tag=f"xt{mi}",
                                           name=f"xt_sb{mi}")
                             for mi in range(MT)]
                    bias_sb = res_pool.tile([P, OUT], mm_dt)
                    if tuned:
                        for mi in range(MT):
                            nc.gpsimd.dma_start(xt_sb[mi][:, 0:2],
                                                xt_d[mi][:, 0:2])
                        for mi in range(MT):
                            nc.gpsimd.dma_start(xt_sb[mi][:, 2:8],
                                                xt_d[mi][:, 2:8])
                        nc.gpsimd.dma_start(bias_sb[:, 0:nt], bias_d[:, 0:nt])
                        # c-tiers ride SP between wA and wB1 so they beat
                        # B1's 8MB in the FIFO race for the shared device
                        for mi in range(MT):
                            nc.sync.dma_start(xt_sb[mi][:, 8:KT],
                                              xt_d[mi][:, 8:KT])
                    else:
                        for mi in range(MT):
                            nc.gpsimd.dma_start(xt_sb[mi][:], xt_d[mi])
                        nc.gpsimd.dma_start(bias_sb[:], bias_d[:])

                    def evict(ps, mi, col0, cw=nt, dma_eng=None):
                        o_sb = o_pool.tile([P, cw], out_dt)
                        nc.vector.tensor_add(
                            out=o_sb[:], in0=ps[:],
                            in1=bias_sb[:, col0:col0 + cw])
                        (dma_eng or nc.scalar).dma_start(
                            out_d[:, mi, col0:col0 + cw], o_sb[:])

                    # ---- segment A: one k-outer sweep, all mi, nblk=1
                    pss = [ps_pool.tile([P, nt], mybir.dt.float32,
                                        tag="ps", name=f"psA_{j}")
                           for j in range(MT)]
                    for k in range(KT):
                        for mj in range(MT):
                            nc.tensor.matmul(
                                pss[mj][:],
                                lhsT=xt_sb[mj][:, k, :],
                                rhs=wA[:, k, :],
                                start=(k == 0),
                                stop=(k == KT - 1),
                            )
                    for mj in range(MT):
                        evict(pss[mj], mj, 0)

                    # ---- segment B1: k-outer, GM=2, nblk=4 (just-in-time
                    # W stream, no prefetch needed during A)
                    nblk = 4
                    bw = nblk * nt                  # 2048
                    col1 = nt
                    wB1 = w_pool.tile([P, KT, bw], mm_dt, tag="w", name="wB1")
                    for k in range(KT):
                        nc.sync.dma_start(wB1[:, k], w_d[:, k, col1:col1 + bw])
                    if tuned:
                        nc.sync.dma_start(bias_sb[:, col1:col1 + bw],
                                          bias_d[:, col1:col1 + bw])
                    GM = min(2, MT)
                    for g in range(0, MT, GM):
                        gm = min(GM, MT - g)
                        pss = [ps_pool.tile([P, nt], mybir.dt.float32,
                                            tag="ps", name=f"psB1{g}_{j}")
                               for j in range(gm * nblk)]
                        for k in range(KT):
                            for mj in range(gm):
                                for ni in range(nblk):
                                    nc.tensor.matmul(
                                        pss[mj * nblk + ni][:],
                                        lhsT=xt_sb[g + mj][:, k, :],
                                        rhs=wB1[:, k, ni * nt:(ni + 1) * nt],
                                        start=(k == 0),
                                        stop=(k == KT - 1),
                                    )
                        for mj in range(gm):
                            for ni in range(nblk):
                                evict(pss[mj * nblk + ni], g + mj,
                                      col1 + ni * nt)

                    # ---- segments B2, B3: mi-outer (prefetched)
                    for blk in range(2):
                        col_b = col1 + bw + blk * bw
                        w_sb = w_pool.tile([P, KT, bw], mm_dt, tag="w",
                                           name=f"wB{blk + 2}")
                        for k in range(KT):
                            nc.sync.dma_start(
                                w_sb[:, k], w_d[:, k, col_b:col_b + bw])
                        if tuned:
                            nc.sync.dma_start(bias_sb[:, col_b:col_b + bw],
                                              bias_d[:, col_b:col_b + bw])
                        for mi in range(MT):
                            pss = [ps_pool.tile([P, nt], mybir.dt.float32,
                                                tag="ps",
                                                name=f"ps{blk}_{mi}_{j}")
                                   for j in range(nblk)]
                            for k in range(KT):
                                for ni in range(nblk):
                                    nc.tensor.matmul(
                                        pss[ni][:],
                                        lhsT=xt_sb[mi][:, k, :],
                                        rhs=w_sb[:, k, ni * nt:(ni + 1) * nt],
                                        start=(k == 0),
                                        stop=(k == KT - 1),
                                    )
                            for ni in range(nblk):
                                evict(pss[ni], mi, col_b + ni * nt)

                    # ---- segment C: last 1536 cols, nblk=3, mi-outer
                    col_c = col1 + 3 * bw           # 6656
                    cww = OUT - col_c               # 1536
                    wC = w_pool.tile([P, KT, cww], mm_dt, tag="w", name="wC")
                    for k in range(KT):
                        nc.sync.dma_start(wC[:, k], w_d[:, k, col_c:OUT])
                    if tuned:
                        nc.sync.dma_start(bias_sb[:, col_c:OUT],
                                          bias_d[:, col_c:OUT])
                    for mi in range(MT):
                        if mi == MT - 1:
                            # sequential shrinking chains 512/512/256/256
                            w0 = 0
                            for j, cw in enumerate((nt, nt, nt // 2, nt // 2)):
                                ps = ps_pool.tile([P, cw], mybir.dt.float32,
                                                  tag="ps",
                                                  name=f"psC_{mi}_{j}")
                                for k in range(KT):
                                    nc.tensor.matmul(
                                        ps[:],
                                        lhsT=xt_sb[mi][:, k, :],
                                        rhs=wC[:, k, w0:w0 + cw],
                                        start=(k == 0),
                                        stop=(k == KT - 1),
                                    )
                                evict(ps, mi, col_c + w0, cw=cw,
                                      dma_eng=nc.sync if j % 2 == 0 else None)
                                w0 += cw
                            continue
                        pss = [ps_pool.tile([P, nt], mybir.dt.float32,
                                            tag="ps", name=f"psC_{mi}_{j}")
                               for j in range(3)]
                        for k in range(KT):
                            for ni in range(3):
                                nc.tensor.matmul(
                                    pss[ni][:],
                                    lhsT=xt_sb[mi][:, k, :],
                                    rhs=wC[:, k, ni * nt:(ni + 1) * nt],
                                    start=(k == 0),
                                    stop=(k == KT - 1),
                                )
                        for ni in range(3):
                            evict(pss[ni], mi, col_c + ni * nt)
    nc.compile()
    names = dict(xt=xt_d.name, w=w_d.name, bias=bias_d.name, out=out_d.name)
    return nc, names


def _get5(cap, reps=1, out_dt_name="bfloat16"):
    key = ("v5", cap, reps, out_dt_name)
    if key not in _cache:
        _cache[key] = _build5(cap, reps, out_dt_name)
    return _cache[key]


def _build_ldw(G, reps=1):
    """LDW-exposure microbench: 2048 independent 512-col bf16 matmuls,
    stationary changes every G matmuls (cycling 16 k-slices of one xt
    tile), 8 rotating PSUM banks, start=stop=True each (no chains). One
    eviction at the end so an output exists. Per-MM floor 213.3ns."""
    import contextlib

    import concourse.mybir as mybir
    import concourse.tile as tile
    from concourse import bacc

    mm_dt = mybir.dt.bfloat16
    NMM = 2048

    nc = bacc.Bacc(None, target_bir_lowering=False, debug=False)
    with tile.TileContext(nc) as tc:
        with tc.tile_pool(name="dram", bufs=1, space="DRAM") as dram:
            xt_d = dram.tile((P, 16, P), mm_dt, kind="ExternalInput")
            w_d = dram.tile((P, NTILE), mm_dt, kind="ExternalInput")
            out_d = dram.tile((P, NTILE), mybir.dt.float32,
                              kind="ExternalOutput")
            with tc.tile_pool(name="res", bufs=1) as res_pool, \
                 tc.tile_pool(name="ev", bufs=1) as o_pool, \
                 tc.tile_pool(name="acc", bufs=8, space="PSUM") as ps_pool:
                xt_sb = res_pool.tile([P, 16, P], mm_dt)
                w_sb = res_pool.tile([P, NTILE], mm_dt)
                nc.gpsimd.dma_start(xt_sb[:], xt_d[:])
                nc.gpsimd.dma_start(w_sb[:], w_d[:])
                loop = tc.For_i(0, reps, 1) if reps > 1 else contextlib.nullcontext()
                with loop:
                    ps = None
                    for i in range(NMM):
                        ps = ps_pool.tile([P, NTILE], mybir.dt.float32)
                        nc.tensor.matmul(
                            ps[:],
                            lhsT=xt_sb[:, (i // G) % 16, :],
                            rhs=w_sb[:],
                            start=True, stop=True,
                        )
                    o_sb = o_pool.tile([P, NTILE], mybir.dt.float32)
                    nc.vector.tensor_copy(out=o_sb[:], in_=ps[:])
                    nc.sync.dma_start(out_d[:], o_sb[:])
    nc.compile()
    return nc, dict(xt=xt_d.name, w=w_d.name, out=out_d.name)


def _build_dr(reps=1):
    """DoubleRow throughput microbench: 2048 fp8e4 DoubleRow matmuls,
    lhsT [P,2,128] (2 stationary planes), rhs [P,2,512] (2 moving
    planes), out [128,512]. Per-MM: 107ns if DoubleRow is 4x bf16
    (cost-model claim), 213ns if 2x (docs claim)."""
    import contextlib

    import concourse.mybir as mybir
    import concourse.tile as tile
    from concourse import bacc

    dt8 = mybir.dt.float8e4
    NMM = 2048

    nc = bacc.Bacc(None, target_bir_lowering=False, debug=False)
    with tile.TileContext(nc) as tc:
        with tc.tile_pool(name="dram", bufs=1, space="DRAM") as dram:
            xt_d = dram.tile((P, 2, 16, P), dt8, kind="ExternalInput")
            w_d = dram.tile((P, 2, NTILE), dt8, kind="ExternalInput")
            out_d = dram.tile((P, NTILE), mybir.dt.float32,
                              kind="ExternalOutput")
            with tc.tile_pool(name="res", bufs=1) as res_pool, \
                 tc.tile_pool(name="ev", bufs=1) as o_pool, \
                 tc.tile_pool(name="acc", bufs=8, space="PSUM") as ps_pool:
                xt_sb = res_pool.tile([P, 2, 16, P], dt8)
                w_sb = res_pool.tile([P, 2, NTILE], dt8)
                nc.gpsimd.dma_start(xt_sb[:], xt_d[:])
                nc.gpsimd.dma_start(w_sb[:], w_d[:])
                loop = tc.For_i(0, reps, 1) if reps > 1 else contextlib.nullcontext()
                with loop:
                    ps = None
                    for i in range(NMM):
                        ps = ps_pool.tile([P, NTILE], mybir.dt.float32)
                        nc.tensor.matmul(
                            ps[:],
                            lhsT=xt_sb[:, :, (i // 4) % 16, :],
                            rhs=w_sb[:],
                            start=True, stop=True,
                            perf_mode=mybir.MatmulPerfMode.DoubleRow,
                        )
                    o_sb = o_pool.tile([P, NTILE], mybir.dt.float32)
                    nc.vector.tensor_copy(out=o_sb[:], in_=ps[:])
                    nc.sync.dma_start(out_d[:], o_sb[:])
    nc.compile()
    return nc, dict(xt=xt_d.name, w=w_d.name, out=out_d.name)


def _get_dr(reps=1):
    key = ("dr", reps)
    if key not in _cache:
        _cache[key] = _build_dr(reps)
    return _cache[key]


def _get_ldw(G, reps=1):
    key = ("ldw", G, reps)
    if key not in _cache:
        _cache[key] = _build_ldw(G, reps)
    return _cache[key]


def _get3(cap, reps=1, out_dt_name="bfloat16"):
    key = ("v3", cap, reps, out_dt_name)
    if key not in _cache:
        _cache[key] = _build3(cap, reps, out_dt_name)
    return _cache[key]


def kernel(inputs, weight, group_sizes, bias):
    import ml_dtypes

    from concourse.bass_utils import run_bass_kernel_spmd

    M = inputs.shape[0]
    gs = np.asarray(group_sizes, dtype=np.int64)
    # per-token expert id exactly as the reference's jnp.repeat(...,
    # total_repeat_length=M): truncate or pad with the last expert id
    ids = np.repeat(np.arange(E), gs)
    ids = ids[:M] if len(ids) >= M else np.concatenate(
        [ids, np.full(M - len(ids), E - 1)])
    counts = np.bincount(ids, minlength=E)
    starts = np.concatenate([[0], np.cumsum(counts)])[:E]

    cap = max(P, int(-(-counts.max() // P) * P))
    nc, names = _get4(cap)

    x = np.asarray(inputs, dtype=np.float32).astype(ml_dtypes.bfloat16)
    w = np.asarray(weight, dtype=np.float32).astype(ml_dtypes.bfloat16)
    bias_rep = np.ascontiguousarray(np.broadcast_to(
        np.asarray(bias, np.float32).astype(ml_dtypes.bfloat16), (P, OUT)))

    in_maps = []
    for e in range(E):
        xe = x[starts[e]:starts[e] + counts[e]]
        if xe.shape[0] < cap:
            xe = np.concatenate(
                [xe, np.zeros((cap - xe.shape[0], IN), ml_dtypes.bfloat16)])
        # [cap, IN] -> (MT, P, KT, P): xt[mi, p, k, j] = X[mi*P+j, k*P+p]
        xt = np.ascontiguousarray(
            xe.reshape(cap // P, P, IN // P, P).transpose(0, 3, 2, 1))
        # [IN, OUT] -> (P, KT, OUT): wt[p, a, n] = W[a*P+p, n]
        we = np.ascontiguousarray(
            w[e].reshape(IN // P, P, OUT).transpose(1, 0, 2))
        in_maps.append({names["xt"]: xt, names["w"]: we,
                        names["bias"]: bias_rep})

    res = run_bass_kernel_spmd(nc, in_maps, core_ids=list(range(E)))
    out = np.empty((M, OUT), dtype=np.float32)
    for e in range(E):
        oe = res.results[e][names["out"]]          # (P, cap//P, OUT) bf16
        oe = oe.astype(np.float32).transpose(1, 0, 2).reshape(cap, OUT)
        out[starts[e]:starts[e] + counts[e]] = oe[:counts[e]]
    return out



# revision 36
# speedup vs baseline: 1.0593x; 1.0593x over previous
"""MoE grouped-linear (ragged matmul + bias) on 8 TRN2 NeuronCores.

Expert-parallel sharding: core e computes tokens of expert e:
    out_e = X_e[cap, 2048] @ W_e[2048, 8192] + bias
Tokens are pre-sorted by expert (contiguous groups), so the "all-to-all"
is a free host-side slice/concat. No on-device collectives.

Production path: _build4 (A/B/C segments), bf16 matmuls, bf16 bias and
bf16 outputs (rel err 3.9e-3 vs the 2e-2 gate). Structure, from the
TimelineSim cost-model (tsim_tool.py) + HW reps-slope measurements:

- Per-core tensor floor = 1,048,576 PE cycles = 436.9us at 2.4GHz; the
  graded baseline (452.2us) was floor + cold-start DMA stalls + drain.
- Segment A (cols 0..1023): k-outer over mi-groups of 4, nblk=2 -> the
  first sweep consumes W at ~150GB/s (vs mi-outer's 600 demand against
  ~352GB/s supply), killing the block-0 stall. HW G-sweep microbench
  (_build_ldw) showed bf16 LDWEIGHTS is fully hidden at G=1/2/4 (the
  documented 107ns exposure was fp32r-specific), so nblk=2 is free.
- Segments B1..B3 (cols 1024..7167, bw=2048, nblk=4, mi-outer): W for
  block b+1 prefetches into the second w_pool buffer during block b.
- Segment C (cols 7168..8191, nblk=2): last mi runs chains of width
  512/256/128/128 sequentially so the drain is one [P,128] evict + 32KB
  DMA (~3.4us vs 5.6 for 4 simultaneous chains). DMA cannot read PSUM
  (SBUF/DRAM only), so the DVE eviction step is irreducible.
- Cold window: A-era loads ride the SP queue in one deadline-ordered
  FIFO (W k-slices just-in-time at 1.71us cadence, xt tiers and biasA
  interleaved in the slack); xt0a on gpsimd and xt1a..3a on the empty
  Act queue (they transfer in the t~1.5-2.5us device slack, freeing the
  SP sequencer, whose 0.65us/DMA dispatch rate is the cold-window
  limiter). 7 warmup matmuls on the framework's preamble-materialized
  bf16 const AP (stride-0 broadcast, no memset needed) ramp the PE
  clock from t~0.8 so real matmuls run at full speed; warmup count is
  sized to bridge exactly to first-data (~3.9us) -- ending early idles
  the PE and resets the p-state ramp.
- DMA scheduling: the simulator's DMA device round-robins *ready*
  transfers from all queues and HWDGE does NOT hold a queue on a
  descriptor's semaphore waits, so queue back-pressure cannot throttle
  loads. Ordering is done by FIFO position instead: early-critical
  loads (xt0..3 in 3 k-tiers, then xt4..7 + biasA, all ordered by first
  need) on gpsimd; per-block bias chunks ride the SP queue AFTER that
  block's W slices. Out-DMAs ride the Activation queue; o_pool bufs=8
  so evictions never wait on out-DMA completion.
- W k0 is chunked 4x512 cols and xt0..3 split at k=2/8 so the first
  matmul gates on ~190KB (~3.4us cold open, latency-dominated).
- PE warmup matmuls during the cold open are NOT useful: the cold
  window is DMA-bound, so slow-clock (p-state ramp) matmuls there are
  free anyway (verified in-model: warmup was net-neutral).

Dead ends, measured on HW: fp8 e4m3 DoubleRow — _build_dr microbench
(2048 DoubleRow insts, 2-plane lhsT/rhs) times like bf16 (~1.0 cycles
per output row, i.e. 2x MACs via the 2-plane contraction), NOT the
0.5 cyc/row the CoreSim cost model charges (which would be 4x). So a
plane-packed 3-term hi/lo scheme (x_hi@w_hi + x_lo@w_hi + x_hi@w_lo,
~1% rel err) costs 1.5x bf16 — slower, confirming the prior session:
single-pass fp8 4.0e-2 / 2-pass 2.7e-2 both fail the 2e-2 gate and
1.5-pass-equivalent packing does not exist at 2x. fp32->bf16-out adds
~1.5e-3 rel err (gate 2e-2) and halves out-DMA. Under tenant/thermal
contention all variants converge to ~540-585us (power-bound regime);
slope measurements need min-of-many-rounds filtering (see test.py).

Second iteration round (same session): the cold-window DMA races were
eliminated by putting the ENTIRE A-window load sequence on the SP
queue in deadline order (single-queue FIFO is deterministic; the
shared DMA device round-robins across queues, so multi-queue splits
of urgent loads always raced). With the PE then the binding resource
from t~3.5us, a p-state warmup became profitable: DVE memsets a
scratch tile at t~0 and 5 dummy matmuls fill the cold open, so the
3us clock ramp completes before real matmuls (real-MM busy hits the
436.7us floor). Tried and rejected: A at 512 cols nblk=1 all-mi
k-outer (concentrates all 8 xt tiles into the first window - worse);
alternating the cold schedule across SP+Act for 2x dispatch rate
(reintroduces device races - worse).

TimelineSim totals (single shot): _build2 474.9us -> _build4 445.4us
= 0.8 preamble+start + 439.6 busy (incl ~3us warmup; real MMs at the
436.6 floor) + ~1.3 residual dispatch-granularity jitter + 3.4 drain.
HW quiet-window slope ~430-440us for all variants (noise swamps
deltas); the graded single-shot metric should capture the ~30us
modeled improvement's real-HW share (~13-16us).
"""

import numpy as np

E, IN, OUT = 8, 2048, 8192
P = 128
NTILE = 512

_cache = {}


def _build(cap, dtype_name="float32r", reps=1, mode="full", ntile=None):
    import contextlib

    import concourse.mybir as mybir
    import concourse.tile as tile
    from concourse import bacc

    mm_dt = getattr(mybir.dt, dtype_name)
    nt = ntile or NTILE
    KT = IN // P            # 16 k-tiles
    MT = cap // P           # m-tiles per core
    NT = OUT // nt          # n-tiles

    nc = bacc.Bacc(None, target_bir_lowering=False, debug=False)
    with tile.TileContext(nc) as tc:
        with tc.tile_pool(name="dram", bufs=1, space="DRAM") as dram:
            # xt_d[mi, p, k, j] = X[mi*P + j, k*P + p] — per-mi contiguous
            # 1MB slices so the first matmul group can start after ~1MB of DMA
            xt_d = dram.tile((MT, P, KT, P), mm_dt, kind="ExternalInput")
            w_d = dram.tile((P, KT, OUT), mm_dt, kind="ExternalInput")
            bias_d = dram.tile((P, OUT), mybir.dt.float32, kind="ExternalInput")
            out_d = dram.tile((P, MT, OUT), mybir.dt.float32, kind="ExternalOutput")

            with tc.tile_pool(name="resident", bufs=1) as res_pool, \
                 tc.tile_pool(name="wchunk", bufs=2) as w_pool, \
                 tc.tile_pool(name="evict", bufs=6) as o_pool, \
                 tc.tile_pool(name="acc", bufs=(3 if nt > 512 else 6), space="PSUM") as ps_pool:
                loop = tc.For_i(0, reps, 1) if reps > 1 else contextlib.nullcontext()
                with loop:
                    # W stream owns the sync queue; X^T + bias load in
                    # parallel on the gpsimd queue, first-needed first.
                    w_sbs = [None] * NT
                    w_sbs[0] = w_pool.tile([P, KT, nt], mm_dt, tag="w",
                                           name="w_sb0")
                    nc.sync.dma_start(w_sbs[0][:], w_d[:, :, 0:nt])

                    xt_sb = [res_pool.tile([P, KT, P], mm_dt, tag=f"xt{mi}",
                                           name=f"xt_sb{mi}")
                             for mi in range(MT)]
                    bias_sb = res_pool.tile([P, OUT], mybir.dt.float32)
                    nc.gpsimd.dma_start(xt_sb[0][:], xt_d[0])
                    nc.gpsimd.dma_start(bias_sb[:], bias_d[:])
                    for mi in range(1, MT):
                        nc.gpsimd.dma_start(xt_sb[mi][:], xt_d[mi])

                    for ni in range(NT):
                        w_sb = w_sbs[ni]
                        if w_sb is None and mode in ("mm_only", "same_w"):
                            w_sb = w_sbs[0]
                        elif w_sb is None:
                            w_sb = w_pool.tile([P, KT, nt], mm_dt, tag="w",
                                               name=f"w_sb{ni}")
                            nc.sync.dma_start(
                                w_sb[:], w_d[:, :, ni * nt:(ni + 1) * nt])
                        for mi in range(MT):
                            ps = ps_pool.tile([P, nt], mybir.dt.float32)
                            for k in range(KT):
                                nc.tensor.matmul(
                                    ps[:],
                                    lhsT=xt_sb[0][:, 0, :] if mode == "same_w"
                                    else xt_sb[mi][:, k, :],
                                    rhs=w_sb[:, k, :],
                                    start=(k == 0),
                                    stop=(k == KT - 1),
                                )
                            if mode in ("mm_only", "same_w") and not (ni == NT - 1 and mi == MT - 1):
                                continue
                            o_sb = o_pool.tile([P, nt], mybir.dt.float32)
                            nc.vector.tensor_add(
                                out=o_sb[:], in0=ps[:],
                                in1=bias_sb[:, ni * nt:(ni + 1) * nt])
                            nc.sync.dma_start(
                                out_d[:, mi, ni * nt:(ni + 1) * nt], o_sb[:])
    nc.compile()
    names = dict(xt=xt_d.name, w=w_d.name, bias=bias_d.name, out=out_d.name)
    return nc, names


def _get(cap, dtype_name="float32r", reps=1, mode="full", ntile=None):
    key = (cap, dtype_name, reps, mode, ntile)
    if key not in _cache:
        _cache[key] = _build(cap, dtype_name, reps, mode, ntile)
    return _cache[key]


def _build2(cap, dtype_name="bfloat16", reps=1, nblk=4, psum_bufs=8,
            mode="full"):
    """LDW-amortized variant: loop (ni_blk, mi, k, ni-in-blk) so each
    stationary x^T[mi,k] serves `nblk` consecutive 512-col matmuls.
    W is streamed once, in [128, KT, nblk*512] blocks, per-k-slice DMAs.
    mode: full | no_evict (only last gen evicts) | same_w (fixed stationary)
    """
    import contextlib

    import concourse.mybir as mybir
    import concourse.tile as tile
    from concourse import bacc

    mm_dt = getattr(mybir.dt, dtype_name)
    nt = NTILE                      # 512
    KT = IN // P                    # 16
    MT = cap // P                   # m-tiles
    NBLK = OUT // (nblk * nt)       # blocks of nblk n-tiles
    bw = nblk * nt                  # block width in cols

    nc = bacc.Bacc(None, target_bir_lowering=False, debug=False)
    with tile.TileContext(nc) as tc:
        with tc.tile_pool(name="dram", bufs=1, space="DRAM") as dram:
            xt_d = dram.tile((MT, P, KT, P), mm_dt, kind="ExternalInput")
            w_d = dram.tile((P, KT, OUT), mm_dt, kind="ExternalInput")
            bias_d = dram.tile((P, OUT), mybir.dt.float32, kind="ExternalInput")
            out_d = dram.tile((P, MT, OUT), mybir.dt.float32, kind="ExternalOutput")

            with tc.tile_pool(name="resident", bufs=1) as res_pool, \
                 tc.tile_pool(name="wblk", bufs=2) as w_pool, \
                 tc.tile_pool(name="evict", bufs=4) as o_pool, \
                 tc.tile_pool(name="acc", bufs=psum_bufs, space="PSUM") as ps_pool:
                loop = tc.For_i(0, reps, 1) if reps > 1 else contextlib.nullcontext()
                with loop:
                    w_sbs = [None] * NBLK
                    w_sbs[0] = w_pool.tile([P, KT, bw], mm_dt, tag="w",
                                           name="w_sb0")
                    # per-k-slice DMAs so the first matmul is gated on
                    # one k-slice, not the whole 8MB block
                    for k in range(KT):
                        nc.sync.dma_start(w_sbs[0][:, k], w_d[:, k, 0:bw])

                    xt_sb = [res_pool.tile([P, KT, P], mm_dt, tag=f"xt{mi}",
                                           name=f"xt_sb{mi}")
                             for mi in range(MT)]
                    bias_sb = res_pool.tile([P, OUT], mybir.dt.float32)
                    nc.gpsimd.dma_start(xt_sb[0][:], xt_d[0])
                    nc.gpsimd.dma_start(bias_sb[:], bias_d[:])
                    for mi in range(1, MT):
                        nc.gpsimd.dma_start(xt_sb[mi][:], xt_d[mi])

                    for blk in range(NBLK):
                        w_sb = w_sbs[blk]
                        if w_sb is None:
                            w_sb = w_pool.tile([P, KT, bw], mm_dt, tag="w",
                                               name=f"w_sb{blk}")
                            for k in range(KT):
                                nc.sync.dma_start(
                                    w_sb[:, k],
                                    w_d[:, k, blk * bw:(blk + 1) * bw])
                        for mi in range(MT):
                            pss = [ps_pool.tile([P, nt], mybir.dt.float32,
                                                tag="ps",
                                                name=f"ps{blk}_{mi}_{j}")
                                   for j in range(nblk)]
                            for k in range(KT):
                                for ni in range(nblk):
                                    nc.tensor.matmul(
                                        pss[ni][:],
                                        lhsT=xt_sb[0][:, 0, :] if mode == "same_w"
                                        else xt_sb[mi][:, k, :],
                                        rhs=w_sb[:, k, ni * nt:(ni + 1) * nt],
                                        start=(k == 0),
                                        stop=(k == KT - 1),
                                    )
                            if mode in ("no_evict", "same_w") and not (
                                    blk == NBLK - 1 and mi == MT - 1):
                                continue
                            for ni in range(nblk):
                                o_sb = o_pool.tile([P, nt], mybir.dt.float32)
                                col0 = blk * bw + ni * nt
                                nc.vector.tensor_add(
                                    out=o_sb[:], in0=pss[ni][:],
                                    in1=bias_sb[:, col0:col0 + nt])
                                nc.sync.dma_start(
                                    out_d[:, mi, col0:col0 + nt], o_sb[:])
    nc.compile()
    names = dict(xt=xt_d.name, w=w_d.name, bias=bias_d.name, out=out_d.name)
    return nc, names


def _get2(cap, dtype_name="bfloat16", reps=1, nblk=4, psum_bufs=8,
          mode="full"):
    key = ("v2", cap, dtype_name, reps, nblk, psum_bufs, mode)
    if key not in _cache:
        _cache[key] = _build2(cap, dtype_name, reps, nblk, psum_bufs, mode)
    return _cache[key]


def _build3(cap, reps=1, out_dt_name="bfloat16"):
    """Cold-start-optimized variant.

    Block A (first 2048 cols): k-outer over mi-pairs so the first sweep
    consumes W k-slices at ~300GB/s (supply ~350) instead of mi-outer's
    600GB/s — kills the block-0 DMA stall. G=4 stationary reuse kept
    (each xt[mi,k] serves ni0..3); 2mi x 4ni = 8 live PSUM banks.
    Blocks B1..B3 (cols 2048..8191): mi-outer as _build2 (prefetched).
    Out DMAs ride the Activation queue so they never block the SP
    queue's W prefetch; non-critical loads (xt2b/3b, xt4..7, biasB) are
    emitted on the Act queue BETWEEN eviction DMAs, so the out-DMAs'
    semaphore waits throttle them until the cold-start window is over.
    bias is bf16 [P, OUT]; outputs are stored bf16 (abs err +<=0.011 vs
    gate 0.114). First matmul gated on ~190KB: W k0 in 4 chunks, xt0/1
    split at k0..1. Tail: last mi's 4 out-DMAs split across SP + Act.
    """
    import contextlib

    import concourse.mybir as mybir
    import concourse.tile as tile
    from concourse import bacc

    mm_dt = mybir.dt.bfloat16
    out_dt = getattr(mybir.dt, out_dt_name)
    nt = NTILE                      # 512
    KT = IN // P                    # 16
    MT = cap // P                   # m-tiles
    nblk = 4
    bw = nblk * nt                  # 2048
    NBLK = OUT // bw                # 4 (A + 3 B-blocks)
    GM = min(2, MT)                 # mi-group size in block A
    tuned = MT == 8                 # DMA schedule tuned for cap=1024

    nc = bacc.Bacc(None, target_bir_lowering=False, debug=False)
    with tile.TileContext(nc) as tc:
        with tc.tile_pool(name="dram", bufs=1, space="DRAM") as dram:
            xt_d = dram.tile((MT, P, KT, P), mm_dt, kind="ExternalInput")
            w_d = dram.tile((P, KT, OUT), mm_dt, kind="ExternalInput")
            bias_d = dram.tile((P, OUT), mm_dt, kind="ExternalInput")
            out_d = dram.tile((P, MT, OUT), out_dt, kind="ExternalOutput")

            with tc.tile_pool(name="resident", bufs=1) as res_pool, \
                 tc.tile_pool(name="wblk", bufs=2) as w_pool, \
                 tc.tile_pool(name="evict", bufs=8) as o_pool, \
                 tc.tile_pool(name="acc", bufs=8, space="PSUM") as ps_pool:
                loop = tc.For_i(0, reps, 1) if reps > 1 else contextlib.nullcontext()
                with loop:
                    # ---- block A weights: per-k DMAs, k0 chunked x4,
                    # k1 chunked x2 so the first matmuls gate on 128KB
                    wA = w_pool.tile([P, KT, bw], mm_dt, tag="w", name="wA")
                    for c in range(4):
                        nc.sync.dma_start(wA[:, 0, c * nt:(c + 1) * nt],
                                          w_d[:, 0, c * nt:(c + 1) * nt])
                    for c in range(2):
                        nc.sync.dma_start(
                            wA[:, 1, c * 2 * nt:(c + 1) * 2 * nt],
                            w_d[:, 1, c * 2 * nt:(c + 1) * 2 * nt])
                    for k in range(2, KT):
                        nc.sync.dma_start(wA[:, k], w_d[:, k, 0:bw])

                    # ---- early-critical loads on the gpsimd queue
                    xt_sb = [res_pool.tile([P, KT, P], mm_dt, tag=f"xt{mi}",
                                           name=f"xt_sb{mi}")
                             for mi in range(MT)]
                    bias_sb = res_pool.tile([P, OUT], mm_dt)
                    if tuned:
                        for mi in (0, 1):
                            nc.gpsimd.dma_start(xt_sb[mi][:, 0:2],
                                                xt_d[mi][:, 0:2])
                        for mi in (0, 1):
                            nc.gpsimd.dma_start(xt_sb[mi][:, 2:KT],
                                                xt_d[mi][:, 2:KT])
                        nc.gpsimd.dma_start(bias_sb[:, 0:bw], bias_d[:, 0:bw])
                        # later-needed xt tiles ride the SP queue AFTER
                        # block A's W slices: FIFO keeps them off the
                        # cold-start window; B1's prefetch has 80us slack
                        for mi in range(2, MT):
                            nc.sync.dma_start(xt_sb[mi][:], xt_d[mi])
                    else:
                        for mi in range(MT):
                            nc.gpsimd.dma_start(xt_sb[mi][:], xt_d[mi])
                        nc.gpsimd.dma_start(bias_sb[:], bias_d[:])

                    def evict(ps, mi, col0, dma_eng=None):
                        o_sb = o_pool.tile([P, nt], out_dt)
                        nc.vector.tensor_add(
                            out=o_sb[:], in0=ps[:],
                            in1=bias_sb[:, col0:col0 + nt])
                        (dma_eng or nc.scalar).dma_start(
                            out_d[:, mi, col0:col0 + nt], o_sb[:])

                    # ---- block A: k-outer over mi-groups
                    for g in range(0, MT, GM):
                        gm = min(GM, MT - g)
                        pss = [ps_pool.tile([P, nt], mybir.dt.float32,
                                            tag="ps", name=f"psA{g}_{j}")
                               for j in range(gm * nblk)]
                        for k in range(KT):
                            for mj in range(gm):
                                for ni in range(nblk):
                                    nc.tensor.matmul(
                                        pss[mj * nblk + ni][:],
                                        lhsT=xt_sb[g + mj][:, k, :],
                                        rhs=wA[:, k, ni * nt:(ni + 1) * nt],
                                        start=(k == 0),
                                        stop=(k == KT - 1),
                                    )
                        for mj in range(gm):
                            for ni in range(nblk):
                                evict(pss[mj * nblk + ni], g + mj, ni * nt)
                        # throttled loads: queued on Act behind this group's
                        # out-DMAs, so they transfer only after the cold
                        # window; each arrives well before it is needed


                    # ---- blocks B1..B3: mi-outer (W prefetched)
                    for blk in range(1, NBLK):
                        w_sb = w_pool.tile([P, KT, bw], mm_dt, tag="w",
                                           name=f"wB{blk}")
                        for k in range(KT):
                            nc.sync.dma_start(
                                w_sb[:, k], w_d[:, k, blk * bw:(blk + 1) * bw])
                        if tuned:
                            nc.sync.dma_start(
                                bias_sb[:, blk * bw:(blk + 1) * bw],
                                bias_d[:, blk * bw:(blk + 1) * bw])
                        for mi in range(MT):
                            pss = [ps_pool.tile([P, nt], mybir.dt.float32,
                                                tag="ps",
                                                name=f"ps{blk}_{mi}_{j}")
                                   for j in range(nblk)]
                            for k in range(KT):
                                for ni in range(nblk):
                                    nc.tensor.matmul(
                                        pss[ni][:],
                                        lhsT=xt_sb[mi][:, k, :],
                                        rhs=w_sb[:, k, ni * nt:(ni + 1) * nt],
                                        start=(k == 0),
                                        stop=(k == KT - 1),
                                    )
                            last = blk == NBLK - 1 and mi == MT - 1
                            for ni in range(nblk):
                                dq = nc.sync if (last and ni < 2) else None
                                evict(pss[ni], mi, blk * bw + ni * nt,
                                      dma_eng=dq)
    nc.compile()
    names = dict(xt=xt_d.name, w=w_d.name, bias=bias_d.name, out=out_d.name)
    return nc, names


def _build4(cap, reps=1, out_dt_name="bfloat16"):
    """A/B/C-segment variant (requires LDW hidden at G=2, measured on HW).

    A: cols 0..1023, k-outer, GM=4 mi-group, nblk=2 (G=2): W demand
    ~150GB/s in the cold window, 8.25MB of early DMA vs ~9.6MB capacity.
    B1..B3: cols 1024..7167, bw=2048 nblk=4 mi-outer (prefetched).
    C: cols 7168..8191, bw=1024 nblk=2 mi-outer: last mi drains only 2
    chains -> short tail; its 2 out-DMAs split across SP/Act queues.
    """
    import contextlib

    import concourse.mybir as mybir
    import concourse.tile as tile
    from concourse import bacc

    mm_dt = mybir.dt.bfloat16
    out_dt = getattr(mybir.dt, out_dt_name)
    nt = NTILE                      # 512
    KT = IN // P                    # 16
    MT = cap // P
    tuned = MT == 8

    nc = bacc.Bacc(None, target_bir_lowering=False, debug=False)
    with tile.TileContext(nc) as tc:
        with tc.tile_pool(name="dram", bufs=1, space="DRAM") as dram:
            xt_d = dram.tile((MT, P, KT, P), mm_dt, kind="ExternalInput")
            w_d = dram.tile((P, KT, OUT), mm_dt, kind="ExternalInput")
            bias_d = dram.tile((P, OUT), mm_dt, kind="ExternalInput")
            out_d = dram.tile((P, MT, OUT), out_dt, kind="ExternalOutput")

            with tc.tile_pool(name="resident", bufs=1) as res_pool, \
                 tc.tile_pool(name="wblk", bufs=2) as w_pool, \
                 tc.tile_pool(name="evict", bufs=8) as o_pool, \
                 tc.tile_pool(name="acc", bufs=8, space="PSUM") as ps_pool:
                loop = tc.For_i(0, reps, 1) if reps > 1 else contextlib.nullcontext()
                with loop:
                    # ---- segment A weights: [P, KT, 1024], per-k DMAs,
                    # k0 in two 512-col chunks
                    awb = 2 * nt    # 1024
                    wA = w_pool.tile([P, KT, awb], mm_dt, tag="w", name="wA")
                    xt_sb = [res_pool.tile([P, KT, P], mm_dt, tag=f"xt{mi}",
                                           name=f"xt_sb{mi}")
                             for mi in range(MT)]
                    bias_sb = res_pool.tile([P, OUT], mm_dt)

                    def wk(k):
                        nc.sync.dma_start(wA[:, k], w_d[:, k, 0:awb])

                    def xts(mi, k0, k1):
                        nc.sync.dma_start(xt_sb[mi][:, k0:k1],
                                          xt_d[mi][:, k0:k1])

                    if tuned:
                        # Deterministic cold-window schedule: everything on
                        # the SP queue in deadline order (single-queue FIFO
                        # -> no cross-queue round-robin races); only xt0a
                        # loads in parallel on gpsimd for the cold open.
                        # Deadlines: wA k_j at ~3.6+1.71j us; xt[mi] tier
                        # (k0..1 / k2..7 / k8..15) at its first matmul;
                        # xt4..7 by group1 (~31us); biasA by first evict.
                        # xt0a on gpsimd and xt1a..3a on the (empty) Act
                        # queue: they transfer in the t~1.5-2.5us device
                        # slack before any contention, freeing the SP
                        # sequencer (0.65us/DMA dispatch) for W k-slices
                        nc.gpsimd.dma_start(xt_sb[0][:, 0:2], xt_d[0][:, 0:2])
                        for mi in (1, 2, 3):
                            nc.scalar.dma_start(xt_sb[mi][:, 0:2],
                                                xt_d[mi][:, 0:2])
                        for c in range(2):
                            nc.sync.dma_start(
                                wA[:, 0, c * nt:(c + 1) * nt],
                                w_d[:, 0, c * nt:(c + 1) * nt])
                        wk(1)
                        wk(2)
                        xts(0, 2, 8)
                        xts(1, 2, 8)
                        xts(2, 2, 8)
                        xts(3, 2, 8)
                        wk(3)
                        wk(4)
                        xts(4, 0, KT)
                        wk(5)
                        xts(0, 8, KT)
                        wk(6)
                        xts(1, 8, KT)
                        wk(7)
                        xts(2, 8, KT)
                        xts(3, 8, KT)
                        wk(8)
                        xts(5, 0, KT)
                        wk(9)
                        wk(10)
                        nc.sync.dma_start(bias_sb[:, 0:awb], bias_d[:, 0:awb])
                        wk(11)
                        xts(6, 0, KT)
                        wk(12)
                        wk(13)
                        wk(14)
                        wk(15)
                        xts(7, 0, KT)
                    else:
                        for c in range(2):
                            nc.sync.dma_start(
                                wA[:, 0, c * nt:(c + 1) * nt],
                                w_d[:, 0, c * nt:(c + 1) * nt])
                        for k in range(1, KT):
                            wk(k)
                        for mi in range(MT):
                            nc.gpsimd.dma_start(xt_sb[mi][:], xt_d[mi])
                        nc.gpsimd.dma_start(bias_sb[:], bias_d[:])

                    def evict(ps, mi, col0, dma_eng=None):
                        o_sb = o_pool.tile([P, nt], out_dt)
                        nc.vector.tensor_add(
                            out=o_sb[:], in0=ps[:],
                            in1=bias_sb[:, col0:col0 + nt])
                        (dma_eng or nc.scalar).dma_start(
                            out_d[:, mi, col0:col0 + nt], o_sb[:])

                    # PE p-state warmup: dummy matmuls on the framework's
                    # preamble-materialized bf16 const AP (stride-0
                    # broadcast) start right after the preamble barrier --
                    # no memset wait -- so the 3us clock ramp completes
                    # during the cold-open DMA wait.
                    wu0 = nc.const_aps.tensor(1.0, (P, P), mybir.dt.bfloat16)
                    wu1 = nc.const_aps.tensor(1.0, (P, nt), mybir.dt.bfloat16)
                    wu_ps = ps_pool.tile([P, nt], mybir.dt.float32,
                                         tag="ps", name="wu_ps")
                    for _ in range(7):
                        nc.tensor.matmul(
                            wu_ps[:], lhsT=wu0, rhs=wu1,
                            start=True, stop=True, skip_group_check=True)

                    # ---- segment A: k-outer, groups of GM=4 mi, nblk=2
                    GM = min(4, MT)
                    for g in range(0, MT, GM):
                        gm = min(GM, MT - g)
                        pss = [ps_pool.tile([P, nt], mybir.dt.float32,
                                            tag="ps", name=f"psA{g}_{j}")
                               for j in range(gm * 2)]
                        for k in range(KT):
                            for mj in range(gm):
                                for ni in range(2):
                                    nc.tensor.matmul(
                                        pss[mj * 2 + ni][:],
                                        lhsT=xt_sb[g + mj][:, k, :],
                                        rhs=wA[:, k, ni * nt:(ni + 1) * nt],
                                        start=(k == 0),
                                        stop=(k == KT - 1),
                                    )
                        for mj in range(gm):
                            for ni in range(2):
                                evict(pss[mj * 2 + ni], g + mj, ni * nt)

                    # ---- segments B: bw=2048, nblk=4, mi-outer
                    nblk = 4
                    bw = nblk * nt
                    nB = (OUT - 2 * awb) // bw      # 3
                    for blk in range(nB):
                        col_b = awb + blk * bw
                        w_sb = w_pool.tile([P, KT, bw], mm_dt, tag="w",
                                           name=f"wB{blk}")
                        for k in range(KT):
                            nc.sync.dma_start(
                                w_sb[:, k], w_d[:, k, col_b:col_b + bw])
                        if tuned:
                            nc.sync.dma_start(bias_sb[:, col_b:col_b + bw],
                                              bias_d[:, col_b:col_b + bw])
                        for mi in range(MT):
                            pss = [ps_pool.tile([P, nt], mybir.dt.float32,
                                                tag="ps",
                                                name=f"ps{blk}_{mi}_{j}")
                                   for j in range(nblk)]
                            for k in range(KT):
                                for ni in range(nblk):
                                    nc.tensor.matmul(
                                        pss[ni][:],
                                        lhsT=xt_sb[mi][:, k, :],
                                        rhs=w_sb[:, k, ni * nt:(ni + 1) * nt],
                                        start=(k == 0),
                                        stop=(k == KT - 1),
                                    )
                            for ni in range(nblk):
                                evict(pss[ni], mi, col_b + ni * nt)

                    # ---- segment C: last 1024 cols, nblk=2, mi-outer
                    col_c = OUT - awb
                    wC = w_pool.tile([P, KT, awb], mm_dt, tag="w", name="wC")
                    for k in range(KT):
                        nc.sync.dma_start(wC[:, k], w_d[:, k, col_c:col_c + awb])
                    if tuned:
                        nc.sync.dma_start(bias_sb[:, col_c:col_c + awb],
                                          bias_d[:, col_c:col_c + awb])
                    for mi in range(MT):
                        last = mi == MT - 1
                        if last:
                            # sequential chains of shrinking width (512,
                            # 256, 256): each eviction hides under the next
                            # chain's matmuls; the drain is one [P,256]
                            # evict + 64KB DMA. (G=1 LDW is free for bf16
                            # per HW measurement.)
                            w0 = 0
                            for j, cw in enumerate(
                                    (nt, nt // 2, nt // 4, nt // 4)):
                                ps = ps_pool.tile([P, cw], mybir.dt.float32,
                                                  tag="ps",
                                                  name=f"psC_{mi}_{j}")
                                for k in range(KT):
                                    nc.tensor.matmul(
                                        ps[:],
                                        lhsT=xt_sb[mi][:, k, :],
                                        rhs=wC[:, k, w0:w0 + cw],
                                        start=(k == 0),
                                        stop=(k == KT - 1),
                                    )
                                o_sb = o_pool.tile([P, cw], out_dt)
                                nc.vector.tensor_add(
                                    out=o_sb[:], in0=ps[:],
                                    in1=bias_sb[:, col_c + w0:col_c + w0 + cw])
                                (nc.sync if j == 0 else nc.scalar).dma_start(
                                    out_d[:, mi, col_c + w0:col_c + w0 + cw],
                                    o_sb[:])
                                w0 += cw
                            continue
                        pss = [ps_pool.tile([P, nt], mybir.dt.float32,
                                            tag="ps", name=f"psC_{mi}_{j}")
                               for j in range(2)]
                        for k in range(KT):
                            for ni in range(2):
                                nc.tensor.matmul(
                                    pss[ni][:],
                                    lhsT=xt_sb[mi][:, k, :],
                                    rhs=wC[:, k, ni * nt:(ni + 1) * nt],
                                    start=(k == 0),
                                    stop=(k == KT - 1),
                                )
                        for ni in range(2):
                            evict(pss[ni], mi, col_c + ni * nt)
    nc.compile()
    names = dict(xt=xt_d.name, w=w_d.name, bias=bias_d.name, out=out_d.name)
    return nc, names


def _get4(cap, reps=1, out_dt_name="bfloat16"):
    key = ("v4", cap, reps, out_dt_name)
    if key not in _cache:
        _cache[key] = _build4(cap, reps, out_dt_name)
    return _cache[key]


def _build5(cap, reps=1, out_dt_name="bfloat16"):
    """A(512, k-outer, nblk=1, all-mi group) variant.

    G=1 LDW is free for bf16 (HW-measured), so segment A can be one
    512-col k-outer sweep over all 8 mi: W demand 75GB/s (21% device
    duty), leaving 79% for the xt tiers -> no early jitter. B1 is
    k-outer (GM=2, nblk=4, ~300GB/s just-in-time) so it needs no
    prefetch during A. B2/B3 mi-outer (prefetched). C = 1536 cols,
    nblk=3; last mi runs sequential chains 512/512/256/256 so the
    drain is one [P,256] evict + 64KB DMA.
    """
    import contextlib

    import concourse.mybir as mybir
    import concourse.tile as tile
    from concourse import bacc

    mm_dt = mybir.dt.bfloat16
    out_dt = getattr(mybir.dt, out_dt_name)
    nt = NTILE                      # 512
    KT = IN // P                    # 16
    MT = cap // P
    tuned = MT == 8

    nc = bacc.Bacc(None, target_bir_lowering=False, debug=False)
    with tile.TileContext(nc) as tc:
        with tc.tile_pool(name="dram", bufs=1, space="DRAM") as dram:
            xt_d = dram.tile((MT, P, KT, P), mm_dt, kind="ExternalInput")
            w_d = dram.tile((P, KT, OUT), mm_dt, kind="ExternalInput")
            bias_d = dram.tile((P, OUT), mm_dt, kind="ExternalInput")
            out_d = dram.tile((P, MT, OUT), out_dt, kind="ExternalOutput")

            with tc.tile_pool(name="resident", bufs=1) as res_pool, \
                 tc.tile_pool(name="wblk", bufs=2) as w_pool, \
                 tc.tile_pool(name="evict", bufs=8) as o_pool, \
                 tc.tile_pool(name="acc", bufs=8, space="PSUM") as ps_pool:
                loop = tc.For_i(0, reps, 1) if reps > 1 else contextlib.nullcontext()
                with loop:
                    # ---- segment A weights: [P, KT, 512], k0 in 2 chunks
                    wA = w_pool.tile([P, KT, nt], mm_dt, tag="w", name="wA")
                    for c in range(2):
                        nc.sync.dma_start(wA[:, 0, c * 256:(c + 1) * 256],
                                          w_d[:, 0, c * 256:(c + 1) * 256])
                    for k in range(1, KT):
                        nc.sync.dma_start(wA[:, k], w_d[:, k, 0:nt])

                    xt_sb = [res_pool.tile([P, KT, P], mm_dt, tag=f"xt{mi}",
                                           name=f"xt_sb{mi}")
                             for mi in range(MT)]
                    bias_sb = res_pool.tile([P, OUT], mm_dt)
                    if tuned:
                        for mi in range(MT):
                            nc.gpsimd.dma_start(xt_sb[mi][:, 0:2],
                                                xt_d[mi][:, 0:2])
                        for mi in range(MT):
                            nc.gpsimd.dma_start(xt_sb[mi][:, 2:8],
                                                xt_d[mi][:, 2:8])
                        nc.gpsimd.dma_start(bias_sb[:, 0:nt], bias_d[:, 0:nt])
                        # c-tiers ride SP between wA and wB1 so they beat
                        # B1's 8MB in the FIFO race for the shared device
                        for mi in range(MT):
                            nc.sync.dma_start(xt_sb[mi][:, 8:KT],
                                              xt_d[mi][:, 8:KT])
                    else:
                        for mi in range(MT):
                            nc.gpsimd.dma_start(xt_sb[mi][:], xt_d[mi])
                        nc.gpsimd.dma_start(bias_sb[:], bias_d[:])

                    def evict(ps, mi, col0, cw=nt, dma_eng=None):
                        o_sb = o_pool.tile([P, cw], out_dt)
                        nc.vector.tensor_add(
                            out=o_sb[:], in0=ps[:],
                            in1=bias_sb[:, col0:col0 + cw])
                        (dma_eng or nc.scalar).dma_start(
                            out_d[:, mi, col0:col0 + cw], o_sb[:])

                    # ---- segment A: one k-outer sweep, all mi, nblk=1
                    pss = [ps_pool.tile([P, nt], mybir.dt.float32,
                                        tag="ps", name=f"psA_{j}")
                           for j in range(MT)]
                    for k in range(KT):
                        for mj in range(MT):
                            nc.tensor.matmul(
                                pss[mj][:],
                                lhsT=xt_sb[mj][:, k, :],
                                rhs=wA[:, k, :],
                                start=(k == 0),
                                stop=(k == KT - 1),
                            )
                    for mj in range(MT):
                        evict(pss[mj], mj, 0)

                    # ---- segment B1: k-outer, GM=2, nblk=4 (just-in-time
                    # W stream, no prefetch needed during A)
                    nblk = 4
                    bw = nblk * nt                  # 2048
                    col1 = nt
                    wB1 = w_pool.tile([P, KT, bw], mm_dt, tag="w", name="wB1")
                    for k in range(KT):
                        nc.sync.dma_start(wB1[:, k], w_d[:, k, col1:col1 + bw])
                    if tuned:
                        nc.sync.dma_start(bias_sb[:, col1:col1 + bw],
                                          bias_d[:, col1:col1 + bw])
                    GM = min(2, MT)
                    for g in range(0, MT, GM):
                        gm = min(GM, MT - g)
                        pss = [ps_pool.tile([P, nt], mybir.dt.float32,
                                            tag="ps", name=f"psB1{g}_{j}")
                               for j in range(gm * nblk)]
                        for k in range(KT):
                            for mj in range(gm):
                                for ni in range(nblk):
                                    nc.tensor.matmul(
                                        pss[mj * nblk + ni][:],
                                        lhsT=xt_sb[g + mj][:, k, :],
                                        rhs=wB1[:, k, ni * nt:(ni + 1) * nt],
                                        start=(k == 0),
                                        stop=(k == KT - 1),
                                    )
                        for mj in range(gm):
                            for ni in range(nblk):
                                evict(pss[mj * nblk + ni], g + mj,
                                      col1 + ni * nt)

                    # ---- segments B2, B3: mi-outer (prefetched)
                    for blk in range(2):
                        col_b = col1 + bw + blk * bw
                        w_sb = w_pool.tile([P, KT, bw], mm_dt, tag="w",
                                           name=f"wB{blk + 2}")
                        for k in range(KT):
                            nc.sync.dma_start(
                                w_sb[:, k], w_d[:, k, col_b:col_b + bw])
                        if tuned:
                            nc.sync.dma_start(bias_sb[:, col_b:col_b + bw],
                                              bias_d[:, col_b:col_b + bw])
                        for mi in range(MT):
                            pss = [ps_pool.tile([P, nt], mybir.dt.float32,
                                                tag="ps",
                                                name=f"ps{blk}_{mi}_{j}")
                                   for j in range(nblk)]
                            for k in range(KT):
                                for ni in range(nblk):
                                    nc.tensor.matmul(
                                        pss[ni][:],
                                        lhsT=xt_sb[mi][:, k, :],
                                        rhs=w_sb[:, k, ni * nt:(ni + 1) * nt],
                                        start=(k == 0),
                                        stop=(k == KT - 1),
                                    )
                            for ni in range(nblk):
                                evict(pss[ni], mi, col_b + ni * nt)

                    # ---- segment C: last 1536 cols, nblk=3, mi-outer
                    col_c = col1 + 3 * bw           # 6656
                    cww = OUT - col_c               # 1536
                    wC = w_pool.tile([P, KT, cww], mm_dt, tag="w", name="wC")
                    for k in range(KT):
                        nc.sync.dma_start(wC[:, k], w_d[:, k, col_c:OUT])
                    if tuned:
                        nc.sync.dma_start(bias_sb[:, col_c:OUT],
                                          bias_d[:, col_c:OUT])
                    for mi in range(MT):
                        if mi == MT - 1:
                            # sequential shrinking chains 512/512/256/256
                            w0 = 0
                            for j, cw in enumerate((nt, nt, nt // 2, nt // 2)):
                                ps = ps_pool.tile([P, cw], mybir.dt.float32,
                                                  tag="ps",
                                                  name=f"psC_{mi}_{j}")
                                for k in range(KT):
                                    nc.tensor.matmul(
                                        ps[:],
                                        lhsT=xt_sb[mi][:, k, :],
                                        rhs=wC[:, k, w0:w0 + cw],
                                        start=(k == 0),
                                        stop=(k == KT - 1),
                                    )
                                evict(ps, mi, col_c + w0, cw=cw,
                                      dma_eng=nc.sync if j % 2 == 0 else None)
                                w0 += cw
                            continue
                        pss = [ps_pool.tile([P, nt], mybir.dt.float32,
                                            tag="ps", name=f"psC_{mi}_{j}")
                               for j in range(3)]
                        for k in range(KT):
                            for ni in range(3):
                                nc.tensor.matmul(
                                    pss[ni][:],
                                    lhsT=xt_sb[mi][:, k, :],
                                    rhs=wC[:, k, ni * nt:(ni + 1) * nt],
                                    start=(k == 0),
                                    stop=(k == KT - 1),
                                )
                        for ni in range(3):
                            evict(pss[ni], mi, col_c + ni * nt)
    nc.compile()
    names = dict(xt=xt_d.name, w=w_d.name, bias=bias_d.name, out=out_d.name)
    return nc, names


def _get5(cap, reps=1, out_dt_name="bfloat16"):
    key = ("v5", cap, reps, out_dt_name)
    if key not in _cache:
        _cache[key] = _build5(cap, reps, out_dt_name)
    return _cache[key]


def _build_ldw(G, reps=1):
    """LDW-exposure microbench: 2048 independent 512-col bf16 matmuls,
    stationary changes every G matmuls (cycling 16 k-slices of one xt
    tile), 8 rotating PSUM banks, start=stop=True each (no chains). One
    eviction at the end so an output exists. Per-MM floor 213.3ns."""
    import contextlib

    import concourse.mybir as mybir
    import concourse.tile as tile
    from concourse import bacc

    mm_dt = mybir.dt.bfloat16
    NMM = 2048

    nc = bacc.Bacc(None, target_bir_lowering=False, debug=False)
    with tile.TileContext(nc) as tc:
        with tc.tile_pool(name="dram", bufs=1, space="DRAM") as dram:
            xt_d = dram.tile((P, 16, P), mm_dt, kind="ExternalInput")
            w_d = dram.tile((P, NTILE), mm_dt, kind="ExternalInput")
            out_d = dram.tile((P, NTILE), mybir.dt.float32,
                              kind="ExternalOutput")
            with tc.tile_pool(name="res", bufs=1) as res_pool, \
                 tc.tile_pool(name="ev", bufs=1) as o_pool, \
                 tc.tile_pool(name="acc", bufs=8, space="PSUM") as ps_pool:
                xt_sb = res_pool.tile([P, 16, P], mm_dt)
                w_sb = res_pool.tile([P, NTILE], mm_dt)
                nc.gpsimd.dma_start(xt_sb[:], xt_d[:])
                nc.gpsimd.dma_start(w_sb[:], w_d[:])
                loop = tc.For_i(0, reps, 1) if reps > 1 else contextlib.nullcontext()
                with loop:
                    ps = None
                    for i in range(NMM):
                        ps = ps_pool.tile([P, NTILE], mybir.dt.float32)
                        nc.tensor.matmul(
                            ps[:],
                            lhsT=xt_sb[:, (i // G) % 16, :],
                            rhs=w_sb[:],
                            start=True, stop=True,
                        )
                    o_sb = o_pool.tile([P, NTILE], mybir.dt.float32)
                    nc.vector.tensor_copy(out=o_sb[:], in_=ps[:])
                    nc.sync.dma_start(out_d[:], o_sb[:])
    nc.compile()
    return nc, dict(xt=xt_d.name, w=w_d.name, out=out_d.name)


def _build_dr(reps=1):
    """DoubleRow throughput microbench: 2048 fp8e4 DoubleRow matmuls,
    lhsT [P,2,128] (2 stationary planes), rhs [P,2,512] (2 moving
    planes), out [128,512]. Per-MM: 107ns if DoubleRow is 4x bf16
    (cost-model claim), 213ns if 2x (docs claim)."""
    import contextlib

    import concourse.mybir as mybir
    import concourse.tile as tile
    from concourse import bacc

    dt8 = mybir.dt.float8e4
    NMM = 2048

    nc = bacc.Bacc(None, target_bir_lowering=False, debug=False)
    with tile.TileContext(nc) as tc:
        with tc.tile_pool(name="dram", bufs=1, space="DRAM") as dram:
            xt_d = dram.tile((P, 2, 16, P), dt8, kind="ExternalInput")
            w_d = dram.tile((P, 2, NTILE), dt8, kind="ExternalInput")
            out_d = dram.tile((P, NTILE), mybir.dt.float32,
                              kind="ExternalOutput")
            with tc.tile_pool(name="res", bufs=1) as res_pool, \
                 tc.tile_pool(name="ev", bufs=1) as o_pool, \
                 tc.tile_pool(name="acc", bufs=8, space="PSUM") as ps_pool:
                xt_sb = res_pool.tile([P, 2, 16, P], dt8)
                w_sb = res_pool.tile([P, 2, NTILE], dt8)
                nc.gpsimd.dma_start(xt_sb[:], xt_d[:])
                nc.gpsimd.dma_start(w_sb[:], w_d[:])
                loop = tc.For_i(0, reps, 1) if reps > 1 else contextlib.nullcontext()
                with loop:
                    ps = None
                    for i in range(NMM):
                        ps = ps_pool.tile([P, NTILE], mybir.dt.float32)
                        nc.tensor.matmul(
                            ps[:],
                            lhsT=xt_sb[:, :, (i // 4) % 16, :],
                            rhs=w_sb[:],
                            start=True, stop=True,
                            perf_mode=mybir.MatmulPerfMode.DoubleRow,
                        )
                    o_sb = o_pool.tile([P, NTILE], mybir.dt.float32)
                    nc.vector.tensor_copy(out=o_sb[:], in_=ps[:])
                    nc.sync.dma_start(out_d[:], o_sb[:])
    nc.compile()
    return nc, dict(xt=xt_d.name, w=w_d.name, out=out_d.name)


def _get_dr(reps=1):
    key = ("dr", reps)
    if key not in _cache:
        _cache[key] = _build_dr(reps)
    return _cache[key]


def _get_ldw(G, reps=1):
    key = ("ldw", G, reps)
    if key not in _cache:
        _cache[key] = _build_ldw(G, reps)
    return _cache[key]


def _get3(cap, reps=1, out_dt_name="bfloat16"):
    key = ("v3", cap, reps, out_dt_name)
    if key not in _cache:
        _cache[key] = _build3(cap, reps, out_dt_name)
    return _cache[key]


def kernel(inputs, weight, group_sizes, bias):
    import ml_dtypes

    from concourse.bass_utils import run_bass_kernel_spmd

    M = inputs.shape[0]
    gs = np.asarray(group_sizes, dtype=np.int64)
    # per-token expert id exactly as the reference's jnp.repeat(...,
    # total_repeat_length=M): truncate or pad with the last expert id
    ids = np.repeat(np.arange(E), gs)
    ids = ids[:M] if len(ids) >= M else np.concatenate(
        [ids, np.full(M - len(ids), E - 1)])
    counts = np.bincount(ids, minlength=E)
    starts = np.concatenate([[0], np.cumsum(counts)])[:E]

    cap = max(P, int(-(-counts.max() // P) * P))
    nc, names = _get4(cap)

    x = np.asarray(inputs, dtype=np.float32).astype(ml_dtypes.bfloat16)
    w = np.asarray(weight, dtype=np.float32).astype(ml_dtypes.bfloat16)
    bias_rep = np.ascontiguousarray(np.broadcast_to(
        np.asarray(bias, np.float32).astype(ml_dtypes.bfloat16), (P, OUT)))

    in_maps = []
    for e in range(E):
        xe = x[starts[e]:starts[e] + counts[e]]
        if xe.shape[0] < cap:
            xe = np.concatenate(
                [xe, np.zeros((cap - xe.shape[0], IN), ml_dtypes.bfloat16)])
        # [cap, IN] -> (MT, P, KT, P): xt[mi, p, k, j] = X[mi*P+j, k*P+p]
        xt = np.ascontiguousarray(
            xe.reshape(cap // P, P, IN // P, P).transpose(0, 3, 2, 1))
        # [IN, OUT] -> (P, KT, OUT): wt[p, a, n] = W[a*P+p, n]
        we = np.ascontiguousarray(
            w[e].reshape(IN // P, P, OUT).transpose(1, 0, 2))
        in_maps.append({names["xt"]: xt, names["w"]: we,
                        names["bias"]: bias_rep})

    res = run_bass_kernel_spmd(nc, in_maps, core_ids=list(range(E)))
    out = np.empty((M, OUT), dtype=np.float32)
    for e in range(E):
        oe = res.results[e][names["out"]]          # (P, cap//P, OUT) bf16
        oe = oe.astype(np.float32).transpose(1, 0, 2).reshape(cap, OUT)
        out[starts[e]:starts[e] + counts[e]] = oe[:counts[e]]
    return out

